# revision 44
# baseline (speedup 1.0000x reference)
"""MaxMarginLoss Trainium2 kernel (8 NeuronCores, vocab-sharded).

Math (reference):
    out_norm = l2norm(preds^T over D)            [B,S,D]
    voc_norm = l2norm(emb over D)                [V,D]
    tgt      = emb[target]                       [B,S,D]
    d        = out_norm@voc_norm.T - tgt@voc_norm.T
    jmax     = argmax_v d
    loss     = mean_masked(relu(g + cos[jmax] - cos[target]))

Key identity: d = (out_norm - tgt) @ voc_norm.T  -> ONE matmul.  Per-row
positive scaling keeps the argmax, so each device computes
    slab[s,v] = (preds[s] - n_s*tgt[s]) . voc_norm[v]   ( = n_s * d[s,v] )
with NO division on device.  The matmul runs in fp8e4m3 (DoubleRow perf
mode, 2 k-subtiles per instruction) accumulating f32 in PSUM.

The matmul runs q-outer (both k-halves of a row tile share stationary
weights across 4 consecutive instructions), which packs PE ~5% tighter.

Device outputs ONLY per-row block maxes, bf16.  Per half (2048 / 1952
cols of PSUM): DVE reduce_max's the first 512 cols straight from PSUM
(16 maxes of W=32); the scalar engine copies the rest to SBUF as bf16,
which DVE folds 32->16->8->4 with three tensor_max's.  The 4 survivors
per 32-block ship as-is (strided mod-4 subsets -- the host knows), so
DVE skips the last reduce and keeps ~350ns/tile of slack under PE's
~1.8us/tile; the end-of-kernel drain is only ~2.8us.  No DRAM slab, no
gathers, no argmax scan on device.

Input DMA: descriptor gen is ~700ns per dma_start serialized per
engine, and each dma_start lands on ~one queue at 30-60GB/s, so ONLY
the t0-critical set (chunks 0-3 as sequential chunk-blocked DRAM
spans, plus eT pieces for t0-t6) is issued up front across
sync/scalar/gpsimd; the late pieces (chunks 4-7, eT for t7-15) are
pinned behind it with tile_wait_until so they don't steal early
bandwidth.  The vocab shard is exactly 4000 cols (no pad; last chunk
is 416).  PE runs scratch warm-up matmuls (p-state ramp) until the
first chunk lands.  Output ships in 4-tile groups, splitting to
single tiles at the end, alternating sync/gpsimd.

Host combine picks the winning (core, block) per row from the 8x404
block maxes, recomputes that block's <=32 exact dots in numpy (the
act-path blocks are strided subsets of their 32-block), resolves the
exact argmax, and finishes the masked-mean loss.
"""

import os
import sys

import numpy as np

for _p in ("/opt/trn_rl_repo", "/root/.axon_site/_ro/trn_rl_repo"):
    if os.path.isdir(_p) and _p not in sys.path:
        sys.path.insert(0, _p)

import concourse.bass as bass
import concourse.bacc as bacc_mod
import concourse.mybir as mybir
from concourse.tile import TileContext

P = 128
B, S, D, V = 4, 512, 512, 32000
BS = B * S                  # 2048 rows
NCORES = 8
VS = V // NCORES            # 4000 vocab cols per core (exact, no pad)
KC = D // P                 # 4 k-subtiles of the contraction
NT = BS // P                # 16 row tiles
CW = [512] * 7 + [416]      # per-chunk widths, sum = 4000
COFF = [sum(CW[:j]) for j in range(9)]          # chunk col offsets
H0 = 2048                   # half 0 cols (chunks 0-3)
H1 = 1952                   # half 1 cols (chunks 4-7)
W = 32                      # block width for block maxes
DCOLS = 512                 # per-half cols DVE reduces straight from PSUM
DNB = DCOLS // W            # 16 direct blocks
ANB0 = (H0 - DCOLS) // W    # 48 act-path blocks, half 0
ANB1 = (H1 - DCOLS) // W    # 45 act-path blocks, half 1
TB0 = DNB + ANB0            # 64 blocks per tile, half 0
TB1 = DNB + ANB1            # 61 blocks per tile, half 1
# per-tile output cols: direct W=32 maxes (16) + act-path W=8 maxes
TC0 = DNB + 4 * ANB0        # 208, half 0
TC1 = DNB + 4 * ANB1        # 196, half 1
NBC = TC0 + TC1             # 404 blocks per row per core
OUT_COLS = NT * (TC0 + TC1)  # 6464
WARM_N = 24                 # PE warm-up matmuls (p-state ramp)
SCALE_E = 0.125
SCALE_V = 16.0
GAMMA = 0.5

F32 = mybir.dt.float32
BF16 = mybir.dt.bfloat16
F8 = mybir.dt.float8e4

_CACHED = {}


def build_nc():
    nc = bacc_mod.Bacc()

    eT8 = nc.declare_dram_parameter("eT8", [P, KC * BS], F8, isOutput=False)
    voc8 = nc.declare_dram_parameter("voc8", [4 * P, 2048], F8, isOutput=False)
    vocH = nc.declare_dram_parameter("vocH", [P, 4 * H1], F8, isOutput=False)
    o_bm = nc.declare_dram_parameter("o_bm", [P, OUT_COLS], BF16, isOutput=True)

    with TileContext(nc) as tc:
        with (
            tc.tile_pool(name="const", bufs=1) as cpool,
            tc.tile_pool(name="bmp", bufs=4) as bmp,
            tc.tile_pool(name="psp", bufs=2, space="PSUM") as psp,
        ):
            # voc chunk tiles: one full chunk per dma_start -- 2KB
            # per-partition descriptor runs hit ~78GB/s vs ~32GB/s for
            # 1KB halves, so 4 parallel full chunks land sooner.
            vC = [cpool.tile([P, KC * CW[j]], F8, tag=f"vC{j}", name=f"vC{j}")
                  for j in range(4)]
            vCH = cpool.tile([P, 4 * H1], F8, tag="vCH")
            eP0 = cpool.tile([P, 512], F8, tag="eP0")
            eP1a = cpool.tile([P, 512], F8, tag="eP1a")
            eP1b = cpool.tile([P, 1024], F8, tag="eP1b")
            eP2 = cpool.tile([P, 1536], F8, tag="eP2")
            eP3 = cpool.tile([P, 1536], F8, tag="eP3")
            eP4 = cpool.tile([P, 3072], F8, tag="eP4")

            # PE warm-up burst (p-state ramp): scratch matmuls keep PE
            # busy while inputs fly.  Vector is idle early, so it does
            # the memset off the DMA-issuing engines' critical path.
            wU = cpool.tile([P, 512], F8, tag="wU")
            nc.vector.memset(wU, 0.0)
            psw = psp.tile([P, 1536], F32, tag="psB", name="ps_warm")
            for i in range(WARM_N):
                nc.tensor.matmul(
                    psw[:, :128],
                    lhsT=wU[:, 0:256].rearrange("p (k m) -> p k m", k=2),
                    rhs=wU[:, 256:512].rearrange("p (k m) -> p k m", k=2),
                    start=True, stop=True,
                    perf_mode=mybir.MatmulPerfMode.DoubleRow,
                )

            def vsl(j):
                # chunk j is a sequential DRAM block: rows j*P..j*P+P,
                # row stride 2048 == row length (fully contiguous span)
                return voc8[j * P:(j + 1) * P, 0:4 * CW[j]]

            # Input DMA: 3 hwdge engines (sync/scalar/gpsimd) generate
            # descriptors in parallel (~700ns per dma_start, serialized
            # per engine; each dma_start lands on ~one queue at
            # ~40-60GB/s), ordered by when compute consumes each piece.
            # The t0-critical pieces are split across two queues each.
            # per-engine gens serialize (~700ns each) and each dma_start
            # lands on its own queue, so: full chunks in strict need
            # order on scalar/sync; the eT pieces stream via gpsimd.
            # critical set only: t0's chunks + eP0/eP1/eP2.  The late
            # pieces are issued inside the loop after the h0 ships so
            # their transfers queue BEHIND the critical set.
            # partition-halved chunk DMAs: each half keeps full 2KB
            # per-partition descriptors (column splits would halve
            # descriptor size and throughput) but rides its own queue,
            # halving each chunk's landing time.
            def vshalf(j, lo, hi):
                return (vC[j][lo:hi, :], voc8[j * P + lo:j * P + hi, :])

            nc.scalar.dma_start(*vshalf(0, 0, 64))
            nc.sync.dma_start(*vshalf(0, 64, 128))
            nc.gpsimd.dma_start(eP0, eT8[:, 0:512])
            nc.scalar.dma_start(*vshalf(1, 0, 64))
            nc.sync.dma_start(*vshalf(1, 64, 128))
            nc.gpsimd.dma_start(eP1a, eT8[:, 512:1024])
            nc.scalar.dma_start(*vshalf(2, 0, 64))
            nc.sync.dma_start(*vshalf(3, 0, 64))
            nc.gpsimd.dma_start(eP1b, eT8[:, 1024:2048])
            nc.scalar.dma_start(*vshalf(2, 64, 128))
            nc.sync.dma_start(*vshalf(3, 64, 128))
            nc.gpsimd.dma_start(eP2, eT8[:, 2048:3584])

            # lhsT views per eT piece: [p, t_local, k, m] (256B runs)
            eV = [eP0[:].rearrange("p (t k m) -> p t k m", t=1, k=KC),
                  eP1a[:].rearrange("p (t k m) -> p t k m", t=1, k=KC),
                  eP1b[:].rearrange("p (t k m) -> p t k m", t=2, k=KC),
                  eP2[:].rearrange("p (t k m) -> p t k m", t=3, k=KC),
                  eP3[:].rearrange("p (t k m) -> p t k m", t=3, k=KC),
                  eP4[:].rearrange("p (t k m) -> p t k m", t=6, k=KC)]
            PSTART = (0, 1, 2, 4, 7, 10)

            def lhsT_view(t, q):
                piece = next(i for i in (5, 4, 3, 2, 1, 0)
                             if t >= PSTART[i])
                tl = t - PSTART[piece]
                return eV[piece][:, tl, 2 * q:2 * q + 2, :]

            HOFF = [4 * (COFF[j] - COFF[4]) for j in range(4, 9)]

            def rhs_view(j, q):
                if j < 4:
                    src = vC[j][:].rearrange("p (k m) -> p k m", k=KC)
                else:
                    src = vCH[:, HOFF[j - 4]:HOFF[j - 3]].rearrange(
                        "p (k m) -> p k m", k=KC)
                return src[:, 2 * q:2 * q + 2, :]

            def emit_folds(stg, acols, anb, bmq, gb, tcb, h, t):
                sv = stg[:, 0:acols].rearrange("p (b w) -> p b w", w=W)
                fs = bmp.tile([P, ANB0 * 28], BF16, tag="fs", name="fs")
                f1r = fs[:, 0:anb * 16].rearrange("p (b w) -> p b w", w=16)
                f2r = fs[:, ANB0 * 16:ANB0 * 16 + anb * 8].rearrange(
                    "p (b w) -> p b w", w=8)
                f3r = bmq[:, gb + DNB:gb + DNB + anb * 4].rearrange(
                    "p (b w) -> p b w", w=4)
                nc.vector.tensor_max(f1r, sv[:, :, 0:16], sv[:, :, 16:32])
                nc.vector.tensor_max(f2r, f1r[:, :, 0:8], f1r[:, :, 8:16])
                nc.vector.tensor_max(f3r, f2r[:, :, 0:4], f2r[:, :, 4:8])
                # ship block maxes: 4-tile groups, but the final tiles
                # individually so the tail after the last matmul is
                # short (alternating engines to pipeline gen)
                if h == 0:
                    ship = [4] if t % 4 == 3 else []
                else:
                    ship = {3: [4], 7: [4], 11: [4], 13: [2],
                            14: [1], 15: [1]}.get(t, [])
                for span in ship:
                    t0g = t - span + 1
                    base = (t0g * TC0 if h == 0
                            else NT * TC0 + t0g * TC1)
                    off = (t0g % 4) * tcb
                    eng = nc.sync if (h * NT + t) % 2 else nc.gpsimd
                    eng.dma_start(o_bm[:, base:base + span * tcb],
                                  bmq[:, off:off + span * tcb])

            pend = None
            # Phase-major: all row tiles of half 0 first (needs only voc
            # chunks 0-3, so compute starts before chunks 4-7 land).
            for h in range(2):
                anb = ANB0 if h == 0 else ANB1
                acols = anb * W
                tb = DNB + anb
                pair = {}
                for t in range(NT):
                    # Two PSUM tiles per half: DVE reduces psA while Act
                    # copies psB -- separate tiles overlap.
                    if h == 0 and t == 0:
                        # Tiles 0 and 1 interleave chunk-major: while
                        # chunk c+1 is still in flight, PE runs both
                        # tiles' chunk-c matmuls instead of stalling
                        # the in-order stream on tile 0 alone.  PSUM
                        # holds exactly two tile generations (8 banks).
                        ps0 = (psp.tile([P, DCOLS], F32, tag="psA",
                                        name="psA0"),
                               psp.tile([P, 1536], F32, tag="psB",
                                        name="psB0"))
                        ps1 = (psp.tile([P, DCOLS], F32, tag="psA",
                                        name="psA1"),
                               psp.tile([P, 1536], F32, tag="psB",
                                        name="psB1"))
                        pair = {0: ps0, 1: ps1}
                        for c in range(4):
                            for tt, q in ((0, 0), (0, 1), (1, 0), (1, 1)):
                                pA, pB = pair[tt]
                                out = pA[:, 0:512] if c == 0 else \
                                    pB[:, (c - 1) * 512:c * 512]
                                nc.tensor.matmul(
                                    out,
                                    lhsT=lhsT_view(tt, q),
                                    rhs=rhs_view(c, q),
                                    start=(q == 0), stop=(q == 1),
                                    perf_mode=mybir.MatmulPerfMode.DoubleRow,
                                )
                    if h == 0 and t in pair:
                        psA, psB = pair[t]
                    else:
                        psA = psp.tile([P, DCOLS], F32, tag="psA")
                        psB = psp.tile([P, 1536], F32, tag="psB")
                        # q-outer: 4 consecutive matmuls share the same
                        # lhsT (stationary weights); PSUM groups
                        # interleave (start on q=0, stop on q=1 pass).
                        for q, c in ((q, c) for q in range(2)
                                     for c in range(4)):
                            j = h * 4 + c
                            wdt = CW[j]
                            out = psA[:, 0:512] if c == 0 else \
                                psB[:, (c - 1) * 512:(c - 1) * 512 + wdt]
                            nc.tensor.matmul(
                                out,
                                lhsT=lhsT_view(t, q),
                                rhs=rhs_view(j, q),
                                start=(q == 0), stop=(q == 1),
                                perf_mode=mybir.MatmulPerfMode.DoubleRow,
                            )
                    # Readout: DVE direct-reduces psA from PSUM (DVE can
                    # read only ONE PSUM operand, 1 elem/cyc f32); Act
                    # bf16-copies psB, which DVE folds 32->16->8 at bf16
                    # speed and reduce_max's over the last 8.
                    tcb = DNB + 4 * anb
                    if t % 4 == 0:
                        bmq = bmp.tile([P, 4 * TC0], BF16, tag="bmq",
                                       name="bmq")
                    g = t % 4
                    gb = g * tcb
                    nc.vector.reduce_max(
                        bmq[:, gb:gb + DNB],
                        psA[:].rearrange("p (b w) -> p b w", w=W),
                        axis=mybir.AxisListType.X,
                    )
                    stg = bmp.tile([P, 1664], BF16, tag="stg")
                    nc.scalar.copy(stg[:, 0:acols], psB[:, 0:acols])
                    # Folds for the PREVIOUS tile are emitted here: the
                    # DVE stream is in-order, so placing folds(t) before
                    # direct(t+1) would stall DVE on COPY(t) while the
                    # ready direct(t+1) -- which frees psA for tile t+3's
                    # first matmul -- waits behind it.  One-tile deferral
                    # keeps every DVE instruction ready when reached.
                    if pend is not None:
                        emit_folds(*pend)
                    pend = (stg, acols, anb, bmq, gb, tcb, h, t)
                    if h == 0 and t == 3:
                        with tc.tile_wait_until(0.012):
                            nc.gpsimd.dma_start(eP3, eT8[:, 3584:5120])
                            nc.sync.dma_start(eP4, eT8[:, 5120:8192])
                    elif h == 0 and t == 7:
                        with tc.tile_wait_until(0.018):
                            nc.sync.dma_start(
                                vCH[:, HOFF[0]:HOFF[1]],
                                vocH[:, HOFF[0]:HOFF[1]])
                            nc.gpsimd.dma_start(
                                vCH[:, HOFF[1]:HOFF[2]],
                                vocH[:, HOFF[1]:HOFF[2]])
                    elif h == 0 and t == 11:
                        with tc.tile_wait_until(0.024):
                            nc.gpsimd.dma_start(
                                vCH[:, HOFF[2]:HOFF[3]],
                                vocH[:, HOFF[2]:HOFF[3]])
                            nc.sync.dma_start(
                                vCH[:, HOFF[3]:HOFF[4]],
                                vocH[:, HOFF[3]:HOFF[4]])

    return nc


def get_nc():
    if "nc" not in _CACHED:
        _CACHED["nc"] = build_nc()
    return _CACHED["nc"]


def _prep(preds, emb_weight, target):
    preds = np.ascontiguousarray(np.asarray(preds, dtype=np.float32))     # [B,D,S]
    emb = np.ascontiguousarray(np.asarray(emb_weight, dtype=np.float32))  # [V,D]
    tgt_idx = np.asarray(target).astype(np.int64).reshape(-1)             # [BS]

    predsN = np.ascontiguousarray(preds.transpose(0, 2, 1).reshape(BS, D))
    n = np.maximum(np.sqrt((predsN ** 2).sum(axis=1)), 1e-12).astype(np.float32)
    tgtN = emb[tgt_idx]                                                   # [BS,D]
    er = predsN - n[:, None] * tgtN                                       # [BS,D]
    vocn = emb / np.maximum(
        np.sqrt((emb ** 2).sum(axis=1, keepdims=True)), 1e-12)            # [V,D]
    return predsN, n, tgtN, er, vocn, tgt_idx


def make_in_maps(preds, emb_weight, target):
    import ml_dtypes
    _, _, _, er, vocn, _ = _prep(preds, emb_weight, target)

    e8 = ((er.T) * SCALE_E).astype(ml_dtypes.float8_e4m3)                 # [D,BS]
    # [p, t, k, m]: row-tile-major so per-tile DMA slices are contiguous
    eT8 = np.ascontiguousarray(
        e8.reshape(KC, P, NT, P).transpose(1, 2, 0, 3).reshape(P, KC * BS))

    in_maps = []
    for c in range(NCORES):
        v8 = (vocn[c * VS:(c + 1) * VS].T * SCALE_V).astype(
            ml_dtypes.float8_e4m3)                                        # [D,VS]
        v8k = v8.reshape(KC, P, VS)
        # chunks 0-3: [P, 2048B] sequential DRAM blocks (k-major inside);
        # chunks 4-7: one partition-major block fetched as a single big DMA
        voc8 = np.zeros((4 * P, 2048), v8.dtype)
        for j in range(4):
            voc8[j * P:(j + 1) * P, :] = (
                v8k[:, :, COFF[j]:COFF[j + 1]].transpose(1, 0, 2).reshape(
                    P, KC * CW[j]))
        vocH = np.concatenate(
            [v8k[:, :, COFF[j]:COFF[j + 1]].transpose(1, 0, 2).reshape(
                P, KC * CW[j]) for j in range(4, 8)], axis=1)
        in_maps.append({"eT8": eT8, "voc8": voc8,
                        "vocH": np.ascontiguousarray(vocH)})
    return in_maps


def combine(results, preds, emb_weight, target, pad_id):
    predsN, n, tgtN, er, vocn, tgt_idx = _prep(preds, emb_weight, target)

    # o_bm [P, 6464] bf16: [h0: 16 tiles x 208][h1: 16 tiles x 196].
    # Within a tile: [16 direct W=32 maxes][4*anb act-path W=8 maxes].
    # Block g of a (half,tile): g<16 covers within-half cols [32g,32g+32);
    # g>=16 covers [512+8(g-16), +8).
    M = np.empty((BS, NCORES, NBC), np.float32)
    for ci, r in enumerate(results):
        a = np.asarray(r["o_bm"]).astype(np.float32)                      # [P,6464]
        a0 = a[:, :NT * TC0].reshape(P, NT, TC0)
        a1 = a[:, NT * TC0:].reshape(P, NT, TC1)
        M[:, ci, :TC0] = a0.transpose(1, 0, 2).reshape(BS, TC0)
        M[:, ci, TC0:] = a1.transpose(1, 0, 2).reshape(BS, TC1)

    win = np.argmax(M.reshape(BS, NCORES * NBC), axis=1)
    core, blk = win // NBC, win % NBC
    h = (blk >= TC0).astype(np.int64)
    g = blk - h * TC0
    # direct block g<16: contiguous [32g, 32g+32).  act block j = g-16:
    # the fold tree's maxes are STRIDED - f3[b, jj] covers cols
    # 512 + 32b + jj + 4k (k<8), where b = j//4, jj = j%4.
    j = g - DNB
    b, jj = j // 4, j % 4
    is_dir = g < DNB
    start = core * VS + h * H0 + np.where(is_dir, 32 * g, 512 + 32 * b + jj)
    step = np.where(is_dir, 1, 4)
    n_c = np.where(is_dir, 32, 8)
    idx = np.arange(W)[None, :]
    cand = start[:, None] + step[:, None] * (idx % n_c[:, None])          # [BS,W]
    cand = np.minimum(cand, V - 1)

    dblk = np.einsum('rd,rwd->rw', er, vocn[cand])
    k = np.argmax(dblk, axis=1)
    jmax = cand[np.arange(BS), k]

    cosmax = (predsN * vocn[jmax]).sum(axis=1) / n
    costgt = (predsN * tgtN).sum(axis=1) / (
        np.maximum(np.sqrt((tgtN ** 2).sum(axis=1)), 1e-12) * n)
    diff = np.maximum(np.float32(GAMMA) + cosmax - costgt, 0.0).astype(np.float32)
    mask = tgt_idx != int(np.asarray(pad_id))
    denom = np.float32(mask.sum())
    loss = np.float32(np.where(mask, diff, np.float32(0.0)).sum() / denom)
    return np.asarray(loss, dtype=np.float32)


def run_cores(in_maps, trace=False):
    from concourse.bass_utils import run_bass_kernel_spmd
    nc = get_nc()
    if not nc.is_finalized():
        nc.finalize()
    return run_bass_kernel_spmd(nc, in_maps, list(range(NCORES)), trace=trace)


def kernel(preds, emb_weight, target, pad_id):
    in_maps = make_in_maps(preds, emb_weight, target)
    res = run_cores(in_maps, trace=False)
    return combine(res.results, preds, emb_weight, target, pad_id)


# revision 46
# speedup vs baseline: 1.0504x; 1.0504x over previous
"""MaxMarginLoss Trainium2 kernel (8 NeuronCores, vocab-sharded).

Math (reference):
    out_norm = l2norm(preds^T over D)            [B,S,D]
    voc_norm = l2norm(emb over D)                [V,D]
    tgt      = emb[target]                       [B,S,D]
    d        = out_norm@voc_norm.T - tgt@voc_norm.T
    jmax     = argmax_v d
    loss     = mean_masked(relu(g + cos[jmax] - cos[target]))

Key identity: d = (out_norm - tgt) @ voc_norm.T  -> ONE matmul.  Per-row
positive scaling keeps the argmax, so each device computes
    slab[s,v] = (preds[s] - n_s*tgt[s]) . voc_norm[v]   ( = n_s * d[s,v] )
with NO division on device.  The matmul runs in fp8e4m3 (DoubleRow perf
mode, 2 k-subtiles per instruction) accumulating f32 in PSUM.

The matmul runs q-outer (both k-halves of a row tile share stationary
weights across 4 consecutive instructions), which packs PE ~5% tighter.

Device outputs ONLY per-row block maxes, bf16.  Per half (2048 / 1952
cols of PSUM): DVE reduce_max's the first 512 cols straight from PSUM
(16 maxes of W=32); the scalar engine copies the rest to SBUF as bf16,
which DVE folds 32->16->8->4 with three tensor_max's.  The 4 survivors
per 32-block ship as-is (strided mod-4 subsets -- the host knows), so
DVE skips the last reduce and keeps ~350ns/tile of slack under PE's
~1.8us/tile; the end-of-kernel drain is only ~2.8us.  No DRAM slab, no
gathers, no argmax scan on device.

Input DMA: descriptor gen is ~700ns per dma_start serialized per
engine, and each dma_start lands on ~one queue at 30-60GB/s, so ONLY
the t0-critical set (chunks 0-3 as sequential chunk-blocked DRAM
spans, plus eT pieces for t0-t6) is issued up front across
sync/scalar/gpsimd; the late pieces (chunks 4-7, eT for t7-15) are
pinned behind it with tile_wait_until so they don't steal early
bandwidth.  The vocab shard is exactly 4000 cols (no pad; last chunk
is 416).  PE runs scratch warm-up matmuls (p-state ramp) until the
first chunk lands.  Output ships in 4-tile groups, splitting to
single tiles at the end, alternating sync/gpsimd.

Host combine picks the winning (core, block) per row from the 8x404
block maxes, recomputes that block's <=32 exact dots in numpy (the
act-path blocks are strided subsets of their 32-block), resolves the
exact argmax, and finishes the masked-mean loss.
"""

import os
import sys

import numpy as np

for _p in ("/opt/trn_rl_repo", "/root/.axon_site/_ro/trn_rl_repo"):
    if os.path.isdir(_p) and _p not in sys.path:
        sys.path.insert(0, _p)

import concourse.bass as bass
import concourse.bacc as bacc_mod
import concourse.mybir as mybir
from concourse.tile import TileContext

P = 128
B, S, D, V = 4, 512, 512, 32000
BS = B * S                  # 2048 rows
NCORES = 8
VS = V // NCORES            # 4000 vocab cols per core (exact, no pad)
KC = D // P                 # 4 k-subtiles of the contraction
NT = BS // P                # 16 row tiles
CW = [512] * 7 + [416]      # per-chunk widths, sum = 4000
COFF = [sum(CW[:j]) for j in range(9)]          # chunk col offsets
H0 = 2048                   # half 0 cols (chunks 0-3)
H1 = 1952                   # half 1 cols (chunks 4-7)
W = 32                      # block width for block maxes
DCOLS = 512                 # per-half cols DVE reduces straight from PSUM
DNB = DCOLS // W            # 16 direct blocks
ANB0 = (H0 - DCOLS) // W    # 48 act-path blocks, half 0
ANB1 = (H1 - DCOLS) // W    # 45 act-path blocks, half 1
TB0 = DNB + ANB0            # 64 blocks per tile, half 0
TB1 = DNB + ANB1            # 61 blocks per tile, half 1
# per-tile output cols: direct W=32 maxes (16) + act-path W=8 maxes
TC0 = DNB + 4 * ANB0        # 208, half 0
TC1 = DNB + 4 * ANB1        # 196, half 1
NBC = TC0 + TC1             # 404 blocks per row per core
OUT_COLS = NT * (TC0 + TC1)  # 6464
WARM_N = 24                 # PE warm-up matmuls (p-state ramp)
SCALE_E = 0.125
SCALE_V = 16.0
GAMMA = 0.5

F32 = mybir.dt.float32
BF16 = mybir.dt.bfloat16
F8 = mybir.dt.float8e4

_CACHED = {}


def build_nc():
    nc = bacc_mod.Bacc()

    eT8 = nc.declare_dram_parameter("eT8", [P, KC * BS], F8, isOutput=False)
    voc8 = nc.declare_dram_parameter("voc8", [4 * P, 2048], F8, isOutput=False)
    vocH = nc.declare_dram_parameter("vocH", [P, 4 * H1], F8, isOutput=False)
    o_bm = nc.declare_dram_parameter("o_bm", [P, OUT_COLS], BF16, isOutput=True)

    with TileContext(nc) as tc:
        with (
            tc.tile_pool(name="const", bufs=1) as cpool,
            tc.tile_pool(name="bmp", bufs=3) as bmp,
            tc.tile_pool(name="psp", bufs=2, space="PSUM") as psp,
        ):
            # voc chunk tiles: one full chunk per dma_start -- 2KB
            # per-partition descriptor runs hit ~78GB/s vs ~32GB/s for
            # 1KB halves, so 4 parallel full chunks land sooner.
            vC = [cpool.tile([P, KC * CW[j]], F8, tag=f"vC{j}", name=f"vC{j}")
                  for j in range(4)]
            vCH = cpool.tile([P, 4 * H1], F8, tag="vCH")
            eP0 = cpool.tile([P, 512], F8, tag="eP0")
            eP1a = cpool.tile([P, 512], F8, tag="eP1a")
            eP1b = cpool.tile([P, 1024], F8, tag="eP1b")
            eP2 = cpool.tile([P, 1536], F8, tag="eP2")
            eP3 = cpool.tile([P, 1536], F8, tag="eP3")
            eP4 = cpool.tile([P, 3072], F8, tag="eP4")

            # PE warm-up burst (p-state ramp): scratch matmuls keep PE
            # busy while inputs fly.  Vector is idle early, so it does
            # the memset off the DMA-issuing engines' critical path.
            wU = cpool.tile([P, 512], F8, tag="wU")
            nc.vector.memset(wU, 0.0)
            psw = psp.tile([P, 1536], F32, tag="psB", name="ps_warm")
            for i in range(WARM_N):
                nc.tensor.matmul(
                    psw[:, :128],
                    lhsT=wU[:, 0:256].rearrange("p (k m) -> p k m", k=2),
                    rhs=wU[:, 256:512].rearrange("p (k m) -> p k m", k=2),
                    start=True, stop=True,
                    perf_mode=mybir.MatmulPerfMode.DoubleRow,
                )

            def vsl(j):
                # chunk j is a sequential DRAM block: rows j*P..j*P+P,
                # row stride 2048 == row length (fully contiguous span)
                return voc8[j * P:(j + 1) * P, 0:4 * CW[j]]

            # Input DMA: 3 hwdge engines (sync/scalar/gpsimd) generate
            # descriptors in parallel (~700ns per dma_start, serialized
            # per engine; each dma_start lands on ~one queue at
            # ~40-60GB/s), ordered by when compute consumes each piece.
            # The t0-critical pieces are split across two queues each.
            # per-engine gens serialize (~700ns each) and each dma_start
            # lands on its own queue, so: full chunks in strict need
            # order on scalar/sync; the eT pieces stream via gpsimd.
            # critical set only: t0's chunks + eP0/eP1/eP2.  The late
            # pieces are issued inside the loop after the h0 ships so
            # their transfers queue BEHIND the critical set.
            # partition-halved chunk DMAs: each half keeps full 2KB
            # per-partition descriptors (column splits would halve
            # descriptor size and throughput) but rides its own queue,
            # halving each chunk's landing time.
            def vshalf(j, lo, hi):
                return (vC[j][lo:hi, :], voc8[j * P + lo:j * P + hi, :])

            nc.scalar.dma_start(*vshalf(0, 0, 64))
            nc.sync.dma_start(*vshalf(0, 64, 128))
            nc.gpsimd.dma_start(eP0[0:64, :], eT8[0:64, 0:512])
            nc.scalar.dma_start(*vshalf(1, 0, 64))
            nc.sync.dma_start(*vshalf(1, 64, 128))
            nc.gpsimd.dma_start(eP0[64:128, :], eT8[64:128, 0:512])
            nc.gpsimd.dma_start(eP1a, eT8[:, 512:1024])
            nc.scalar.dma_start(*vshalf(2, 0, 64))
            nc.sync.dma_start(*vshalf(3, 0, 64))
            nc.gpsimd.dma_start(eP1b, eT8[:, 1024:2048])
            nc.scalar.dma_start(*vshalf(2, 64, 128))
            nc.sync.dma_start(*vshalf(3, 64, 128))
            nc.gpsimd.dma_start(eP2, eT8[:, 2048:3584])

            # lhsT views per eT piece: [p, t_local, k, m] (256B runs)
            eV = [eP0[:].rearrange("p (t k m) -> p t k m", t=1, k=KC),
                  eP1a[:].rearrange("p (t k m) -> p t k m", t=1, k=KC),
                  eP1b[:].rearrange("p (t k m) -> p t k m", t=2, k=KC),
                  eP2[:].rearrange("p (t k m) -> p t k m", t=3, k=KC),
                  eP3[:].rearrange("p (t k m) -> p t k m", t=3, k=KC),
                  eP4[:].rearrange("p (t k m) -> p t k m", t=6, k=KC)]
            PSTART = (0, 1, 2, 4, 7, 10)

            def lhsT_view(t, q):
                piece = next(i for i in (5, 4, 3, 2, 1, 0)
                             if t >= PSTART[i])
                tl = t - PSTART[piece]
                return eV[piece][:, tl, 2 * q:2 * q + 2, :]

            HOFF = [4 * (COFF[j] - COFF[4]) for j in range(4, 9)]

            def rhs_view(j, q):
                if j < 4:
                    src = vC[j][:].rearrange("p (k m) -> p k m", k=KC)
                else:
                    src = vCH[:, HOFF[j - 4]:HOFF[j - 3]].rearrange(
                        "p (k m) -> p k m", k=KC)
                return src[:, 2 * q:2 * q + 2, :]

            def emit_folds(stg, acols, anb, bmq, gb, tcb, h, t):
                sv = stg[:, 0:acols].rearrange("p (b w) -> p b w", w=W)
                fs = bmp.tile([P, ANB0 * 28], BF16, tag="fs", name="fs")
                f1r = fs[:, 0:anb * 16].rearrange("p (b w) -> p b w", w=16)
                f2r = fs[:, ANB0 * 16:ANB0 * 16 + anb * 8].rearrange(
                    "p (b w) -> p b w", w=8)
                f3r = bmq[:, gb + DNB:gb + DNB + anb * 4].rearrange(
                    "p (b w) -> p b w", w=4)
                nc.vector.tensor_max(f1r, sv[:, :, 0:16], sv[:, :, 16:32])
                nc.vector.tensor_max(f2r, f1r[:, :, 0:8], f1r[:, :, 8:16])
                nc.vector.tensor_max(f3r, f2r[:, :, 0:4], f2r[:, :, 4:8])
                # ship block maxes: 4-tile groups, but the final tiles
                # individually so the tail after the last matmul is
                # short (alternating engines to pipeline gen)
                if h == 0:
                    ship = [4] if t % 4 == 3 else []
                else:
                    ship = {3: [4], 7: [4], 11: [4], 13: [2],
                            14: [1], 15: [1]}.get(t, [])
                for span in ship:
                    t0g = t - span + 1
                    base = (t0g * TC0 if h == 0
                            else NT * TC0 + t0g * TC1)
                    off = (t0g % 4) * tcb
                    eng = nc.sync if (h * NT + t) % 2 else nc.gpsimd
                    eng.dma_start(o_bm[:, base:base + span * tcb],
                                  bmq[:, off:off + span * tcb])

            pend = None
            # Phase-major: all row tiles of half 0 first (needs only voc
            # chunks 0-3, so compute starts before chunks 4-7 land).
            for h in range(2):
                anb = ANB0 if h == 0 else ANB1
                acols = anb * W
                tb = DNB + anb
                pair = {}
                for t in range(NT):
                    # Two PSUM tiles per half: DVE reduces psA while Act
                    # copies psB -- separate tiles overlap.
                    if h == 0 and t == 0:
                        # Tiles 0 and 1 interleave chunk-major: while
                        # chunk c+1 is still in flight, PE runs both
                        # tiles' chunk-c matmuls instead of stalling
                        # the in-order stream on tile 0 alone.  PSUM
                        # holds exactly two tile generations (8 banks).
                        ps0 = (psp.tile([P, DCOLS], F32, tag="psA",
                                        name="psA0"),
                               psp.tile([P, 1536], F32, tag="psB",
                                        name="psB0"))
                        ps1 = (psp.tile([P, DCOLS], F32, tag="psA",
                                        name="psA1"),
                               psp.tile([P, 1536], F32, tag="psB",
                                        name="psB1"))
                        pair = {0: ps0, 1: ps1}
                        for c in range(4):
                            for tt, q in ((0, 0), (0, 1), (1, 0), (1, 1)):
                                pA, pB = pair[tt]
                                out = pA[:, 0:512] if c == 0 else \
                                    pB[:, (c - 1) * 512:c * 512]
                                nc.tensor.matmul(
                                    out,
                                    lhsT=lhsT_view(tt, q),
                                    rhs=rhs_view(c, q),
                                    start=(q == 0), stop=(q == 1),
                                    perf_mode=mybir.MatmulPerfMode.DoubleRow,
                                )
                    if h == 0 and t in pair:
                        psA, psB = pair[t]
                    else:
                        psA = psp.tile([P, DCOLS], F32, tag="psA")
                        psB = psp.tile([P, 1536], F32, tag="psB")
                        # q-outer: 4 consecutive matmuls share the same
                        # lhsT (stationary weights); PSUM groups
                        # interleave (start on q=0, stop on q=1 pass).
                        for q, c in ((q, c) for q in range(2)
                                     for c in range(4)):
                            j = h * 4 + c
                            wdt = CW[j]
                            out = psA[:, 0:512] if c == 0 else \
                                psB[:, (c - 1) * 512:(c - 1) * 512 + wdt]
                            nc.tensor.matmul(
                                out,
                                lhsT=lhsT_view(t, q),
                                rhs=rhs_view(j, q),
                                start=(q == 0), stop=(q == 1),
                                perf_mode=mybir.MatmulPerfMode.DoubleRow,
                            )
                    # Readout: DVE direct-reduces psA from PSUM (DVE can
                    # read only ONE PSUM operand, 1 elem/cyc f32); Act
                    # bf16-copies psB, which DVE folds 32->16->8 at bf16
                    # speed and reduce_max's over the last 8.
                    tcb = DNB + 4 * anb
                    if t % 4 == 0:
                        bmq = bmp.tile([P, 4 * TC0], BF16, tag="bmq",
                                       name="bmq")
                    g = t % 4
                    gb = g * tcb
                    nc.vector.reduce_max(
                        bmq[:, gb:gb + DNB],
                        psA[:].rearrange("p (b w) -> p b w", w=W),
                        axis=mybir.AxisListType.X,
                    )
                    stg = bmp.tile([P, 1664], BF16, tag="stg")
                    nc.scalar.copy(stg[:, 0:acols], psB[:, 0:acols])
                    # Folds for the PREVIOUS tile are emitted here: the
                    # DVE stream is in-order, so placing folds(t) before
                    # direct(t+1) would stall DVE on COPY(t) while the
                    # ready direct(t+1) -- which frees psA for tile t+3's
                    # first matmul -- waits behind it.  One-tile deferral
                    # keeps every DVE instruction ready when reached.
                    if pend is not None:
                        emit_folds(*pend)
                    pend = (stg, acols, anb, bmq, gb, tcb, h, t)
                    if h == 0 and t == 3:
                        with tc.tile_wait_until(0.012):
                            nc.gpsimd.dma_start(eP3, eT8[:, 3584:5120])
                            nc.sync.dma_start(eP4, eT8[:, 5120:8192])
                    elif h == 0 and t == 7:
                        with tc.tile_wait_until(0.018):
                            nc.sync.dma_start(
                                vCH[:, HOFF[0]:HOFF[1]],
                                vocH[:, HOFF[0]:HOFF[1]])
                            nc.gpsimd.dma_start(
                                vCH[:, HOFF[1]:HOFF[2]],
                                vocH[:, HOFF[1]:HOFF[2]])
                    elif h == 0 and t == 11:
                        with tc.tile_wait_until(0.024):
                            nc.gpsimd.dma_start(
                                vCH[:, HOFF[2]:HOFF[3]],
                                vocH[:, HOFF[2]:HOFF[3]])
                            nc.sync.dma_start(
                                vCH[:, HOFF[3]:HOFF[4]],
                                vocH[:, HOFF[3]:HOFF[4]])

    return nc


def get_nc():
    if "nc" not in _CACHED:
        _CACHED["nc"] = build_nc()
    return _CACHED["nc"]


def _prep(preds, emb_weight, target):
    preds = np.ascontiguousarray(np.asarray(preds, dtype=np.float32))     # [B,D,S]
    emb = np.ascontiguousarray(np.asarray(emb_weight, dtype=np.float32))  # [V,D]
    tgt_idx = np.asarray(target).astype(np.int64).reshape(-1)             # [BS]

    predsN = np.ascontiguousarray(preds.transpose(0, 2, 1).reshape(BS, D))
    n = np.maximum(np.sqrt((predsN ** 2).sum(axis=1)), 1e-12).astype(np.float32)
    tgtN = emb[tgt_idx]                                                   # [BS,D]
    er = predsN - n[:, None] * tgtN                                       # [BS,D]
    vocn = emb / np.maximum(
        np.sqrt((emb ** 2).sum(axis=1, keepdims=True)), 1e-12)            # [V,D]
    return predsN, n, tgtN, er, vocn, tgt_idx


def make_in_maps(preds, emb_weight, target):
    import ml_dtypes
    _, _, _, er, vocn, _ = _prep(preds, emb_weight, target)

    e8 = ((er.T) * SCALE_E).astype(ml_dtypes.float8_e4m3)                 # [D,BS]
    # [p, t, k, m]: row-tile-major so per-tile DMA slices are contiguous
    eT8 = np.ascontiguousarray(
        e8.reshape(KC, P, NT, P).transpose(1, 2, 0, 3).reshape(P, KC * BS))

    in_maps = []
    for c in range(NCORES):
        v8 = (vocn[c * VS:(c + 1) * VS].T * SCALE_V).astype(
            ml_dtypes.float8_e4m3)                                        # [D,VS]
        v8k = v8.reshape(KC, P, VS)
        # chunks 0-3: [P, 2048B] sequential DRAM blocks (k-major inside);
        # chunks 4-7: one partition-major block fetched as a single big DMA
        voc8 = np.zeros((4 * P, 2048), v8.dtype)
        for j in range(4):
            voc8[j * P:(j + 1) * P, :] = (
                v8k[:, :, COFF[j]:COFF[j + 1]].transpose(1, 0, 2).reshape(
                    P, KC * CW[j]))
        vocH = np.concatenate(
            [v8k[:, :, COFF[j]:COFF[j + 1]].transpose(1, 0, 2).reshape(
                P, KC * CW[j]) for j in range(4, 8)], axis=1)
        in_maps.append({"eT8": eT8, "voc8": voc8,
                        "vocH": np.ascontiguousarray(vocH)})
    return in_maps


def combine(results, preds, emb_weight, target, pad_id):
    predsN, n, tgtN, er, vocn, tgt_idx = _prep(preds, emb_weight, target)

    # o_bm [P, 6464] bf16: [h0: 16 tiles x 208][h1: 16 tiles x 196].
    # Within a tile: [16 direct W=32 maxes][4*anb act-path W=8 maxes].
    # Block g of a (half,tile): g<16 covers within-half cols [32g,32g+32);
    # g>=16 covers [512+8(g-16), +8).
    M = np.empty((BS, NCORES, NBC), np.float32)
    for ci, r in enumerate(results):
        a = np.asarray(r["o_bm"]).astype(np.float32)                      # [P,6464]
        a0 = a[:, :NT * TC0].reshape(P, NT, TC0)
        a1 = a[:, NT * TC0:].reshape(P, NT, TC1)
        M[:, ci, :TC0] = a0.transpose(1, 0, 2).reshape(BS, TC0)
        M[:, ci, TC0:] = a1.transpose(1, 0, 2).reshape(BS, TC1)

    win = np.argmax(M.reshape(BS, NCORES * NBC), axis=1)
    core, blk = win // NBC, win % NBC
    h = (blk >= TC0).astype(np.int64)
    g = blk - h * TC0
    # direct block g<16: contiguous [32g, 32g+32).  act block j = g-16:
    # the fold tree's maxes are STRIDED - f3[b, jj] covers cols
    # 512 + 32b + jj + 4k (k<8), where b = j//4, jj = j%4.
    j = g - DNB
    b, jj = j // 4, j % 4
    is_dir = g < DNB
    start = core * VS + h * H0 + np.where(is_dir, 32 * g, 512 + 32 * b + jj)
    step = np.where(is_dir, 1, 4)
    n_c = np.where(is_dir, 32, 8)
    idx = np.arange(W)[None, :]
    cand = start[:, None] + step[:, None] * (idx % n_c[:, None])          # [BS,W]
    cand = np.minimum(cand, V - 1)

    dblk = np.einsum('rd,rwd->rw', er, vocn[cand])
    k = np.argmax(dblk, axis=1)
    jmax = cand[np.arange(BS), k]

    cosmax = (predsN * vocn[jmax]).sum(axis=1) / n
    costgt = (predsN * tgtN).sum(axis=1) / (
        np.maximum(np.sqrt((tgtN ** 2).sum(axis=1)), 1e-12) * n)
    diff = np.maximum(np.float32(GAMMA) + cosmax - costgt, 0.0).astype(np.float32)
    mask = tgt_idx != int(np.asarray(pad_id))
    denom = np.float32(mask.sum())
    loss = np.float32(np.where(mask, diff, np.float32(0.0)).sum() / denom)
    return np.asarray(loss, dtype=np.float32)


def run_cores(in_maps, trace=False):
    from concourse.bass_utils import run_bass_kernel_spmd
    nc = get_nc()
    if not nc.is_finalized():
        nc.finalize()
    return run_bass_kernel_spmd(nc, in_maps, list(range(NCORES)), trace=trace)


def kernel(preds, emb_weight, target, pad_id):
    in_maps = make_in_maps(preds, emb_weight, target)
    res = run_cores(in_maps, trace=False)
    return combine(res.results, preds, emb_weight, target, pad_id)
